# revision 19
# baseline (speedup 1.0000x reference)
"""TRN2 Bass kernel for nn_KNN_model (conv stack + pairwise patch distances).

Strategy (8 NeuronCores, SPMD):
  - Convs sharded over H: each core computes a 40-row slab (32 owned + 4 halo
    each side) through all 4 conv+BN+ReLU layers in float32r (TF32-like) on PE.
    3x3 conv = 6 matmul streams per tile: 3 K=128 pairs (top+mid tap rows via a
    partition-shifted slab copy) + 3 K=64 singles (bottom tap row).
  - BN stats: per-core partial (mean, var) via bn_stats/bn_aggr over owned rows,
    tiny AllGather + PE ones-matmul reduce, scale/shift transposed to
    per-partition vectors via K=1 matmuls; applied fused in one ACT pass
    (relu(scale*y+shift)) that also casts to f32r for the next conv.
  - Out-of-image halo rows are zeroed with a per-core mask input (SPMD-safe).
  - Final features -> patch matrix (16 x 1024 local) via a strided scatter DMA,
    augmented to 18 rows (p, sq, ones), AllGathered; distance block
    D[i,j] = sqrt(relu(sq_i + sq_j - 2 p_i.p_j)) computed as ONE fp32 K=18
    matmul per [128,512] tile, DVE relu, ACT sqrt, 4MB DMAs out.
"""
import numpy as np
import ml_dtypes
import concourse.bacc as bacc
import concourse.bass as bass
import concourse.tile as tile
from concourse import mybir
from concourse.bass_utils import run_bass_kernel_spmd

F32 = mybir.dt.float32
F32R = mybir.dt.float32r
BF16 = mybir.dt.bfloat16
AF = mybir.ActivationFunctionType
ALU = mybir.AluOpType

NCORES = 8
WP = 258            # padded row width (256 + 2 pad cols)
ROWS = 40           # ext slab rows per core (32 owned + 4 halo each side)
LEAD = 4            # lead margin so tap offsets never go negative
HROWS = 42          # slab rows + 1 pad row top/bottom
HFREE = LEAD + HROWS * WP + 4   # 10844
YFREE = ROWS * WP   # 10320
EPS = 1e-5
GOFF = [0, 64, 128, 192]        # g/be packing offsets per layer
COUT = [64, 64, 64, 2]

_CACHE = {}


def _conv_tiles():
    out, s = [], 0
    while s < YFREE:
        L = min(512, YFREE - s)
        out.append((s, L))
        s += L
    return out


def build():
    nc = bacc.Bacc(trn_type="TRN2", num_devices=NCORES)
    x0 = nc.dram_tensor("x0", [27, YFREE], BF16, kind="ExternalInput").ap()
    w0T = nc.dram_tensor("w0T", [27, 64], BF16, kind="ExternalInput").ap()
    wp_in, ws_in = {}, {}
    for l in (1, 2, 3):
        co = COUT[l]
        wp_in[l] = nc.dram_tensor(f"wp{l}", [3, 128, co], BF16, kind="ExternalInput").ap()
        ws_in[l] = nc.dram_tensor(f"ws{l}", [3, 64, co], BF16, kind="ExternalInput").ap()
    g_all = nc.dram_tensor("g_all", [1, 194], F32, kind="ExternalInput").ap()
    be_all = nc.dram_tensor("be_all", [1, 194], F32, kind="ExternalInput").ap()
    mask8 = nc.dram_tensor("mask8", [1, 8 * WP], BF16, kind="ExternalInput").ap()
    out = nc.dram_tensor("out", [1024, 8192], F32, kind="ExternalOutput").ap()

    TILES = _conv_tiles()

    with tile.TileContext(nc) as tc:
      with tc.tile_pool(name="pers", bufs=1) as pers, \
           tc.tile_pool(name="dr", bufs=1, space="DRAM") as dr:
        gsb = pers.tile([1, 194], F32)
        nc.sync.dma_start(out=gsb, in_=g_all)
        besb = pers.tile([1, 194], F32)
        nc.sync.dma_start(out=besb, in_=be_all)
        ones1 = pers.tile([1, 1], F32)
        nc.vector.memset(ones1, 1.0)
        ones8 = pers.tile([8, 1], F32)
        nc.vector.memset(ones8, 0.125)   # 1/8 for mean-of-cores matmul
        epst = pers.tile([1, 1], F32)
        nc.vector.memset(epst, EPS)

        def bn_finish(l, C, regions, bnps, sbp):
            """Cross-core BN: partial stats -> AllGather -> scale/shift [C,1]."""
            # each region is [C, k, <=512]; bn_stats keeps non-innermost dims
            n = sum(r.shape[1] if r.ndim == 3 else 1 for r in regions)
            st = sbp.tile([C, n, 6], F32, tag=f"st{l}")
            i = 0
            for ap in regions:
                k = ap.shape[1] if ap.ndim == 3 else 1
                o = st[:, i:i + k, :] if ap.ndim == 3 else st[:, i, :]
                nc.vector.bn_stats(out=o, in_=ap)
                i += k
            mvt = sbp.tile([C, 2], F32, tag=f"mv{l}")
            nc.vector.bn_aggr(out=mvt, in_=st)
            sti = dr.tile([C, 2], F32, tag=f"sti{l}")
            sto = dr.tile([NCORES, C, 2], F32, tag=f"sto{l}")
            nc.gpsimd.dma_start(out=sti, in_=mvt)
            nc.gpsimd.collective_compute(
                "AllGather", ALU.bypass,
                replica_groups=[list(range(NCORES))],
                ins=[sti.opt()], outs=[sto.opt()])
            G = sbp.tile([8, 2 * C], F32, tag=f"G{l}")
            nc.sync.dma_start(out=G, in_=sto.rearrange("k c two -> k (c two)"))
            Gv = G.rearrange("p (c two) -> p c two", two=2)
            m2 = sbp.tile([8, C], F32, tag=f"m2{l}")
            nc.vector.tensor_mul(m2, Gv[:, :, 0], Gv[:, :, 0])
            pavg = bnps.tile([1, 2 * C], F32, tag="bn")
            nc.tensor.matmul(pavg, ones8, G, start=True, stop=True)
            pavg2 = bnps.tile([1, C], F32, tag="bn")
            nc.tensor.matmul(pavg2, ones8, m2, start=True, stop=True)
            A1 = sbp.tile([1, 2 * C], F32, tag=f"A1{l}")
            nc.scalar.copy(A1, pavg)
            A2 = sbp.tile([1, C], F32, tag=f"A2{l}")
            nc.scalar.copy(A2, pavg2)
            A1v = A1.rearrange("p (c two) -> p c two", two=2)
            am, av = A1v[:, :, 0], A1v[:, :, 1]
            t1 = sbp.tile([1, C], F32, tag=f"t1{l}")
            nc.vector.tensor_mul(t1, am, am)       # E[m]^2
            t2 = sbp.tile([1, C], F32, tag=f"t2{l}")
            nc.vector.tensor_sub(t2, A2, t1)       # Var(means)
            t3 = sbp.tile([1, C], F32, tag=f"t3{l}")
            nc.vector.tensor_add(t3, t2, av)       # + E[var] = total var
            sd = sbp.tile([1, C], F32, tag=f"sd{l}")
            nc.scalar.activation(sd, t3, AF.Sqrt, bias=epst)
            rs = sbp.tile([1, C], F32, tag=f"rs{l}")
            nc.vector.reciprocal(rs, sd)
            off = GOFF[l]
            scl = sbp.tile([1, C], F32, tag=f"scl{l}")
            nc.vector.tensor_mul(scl, gsb[:, off:off + C], rs)
            sh0 = sbp.tile([1, C], F32, tag=f"sh0{l}")
            nc.vector.tensor_mul(sh0, am, scl)
            sh = sbp.tile([1, C], F32, tag=f"sh{l}")
            nc.vector.tensor_sub(sh, besb[:, off:off + C], sh0)
            psc = bnps.tile([C, 1], F32, tag="bn")
            nc.tensor.matmul(psc, scl, ones1, start=True, stop=True)
            psh = bnps.tile([C, 1], F32, tag="bn")
            nc.tensor.matmul(psh, sh, ones1, start=True, stop=True)
            sbs = sbp.tile([C, 1], F32, tag=f"sbs{l}")
            nc.scalar.copy(sbs, psc)
            sbh = sbp.tile([C, 1], F32, tag=f"sbh{l}")
            nc.scalar.copy(sbh, psh)
            return sbs, sbh

        # ---------------- conv phase ----------------
        with tc.tile_pool(name="cb", bufs=1) as cb, \
             tc.tile_pool(name="hp", bufs=2) as hp, \
             tc.tile_pool(name="cps", bufs=6, space="PSUM") as cps, \
             tc.tile_pool(name="bnps", bufs=2, space="PSUM") as bnps:
            mskf = cb.tile([64, 8 * WP], BF16)
            nc.gpsimd.dma_start(out=mskf, in_=mask8.partition_broadcast(64))
            mv_ = mskf.rearrange("p (r c) -> p r c", c=WP)
            w0 = cb.tile([27, 64], BF16)
            nc.sync.dma_start(out=w0, in_=w0T)
            wpair, wsing = {}, {}
            for l in (1, 2, 3):
                co = COUT[l]
                for p in range(3):
                    t = cb.tile([128, co], BF16, tag=f"twp{l}{p}")
                    nc.sync.dma_start(out=t, in_=wp_in[l][p])
                    wpair[(l, p)] = t
                    t2 = cb.tile([64, co], BF16, tag=f"tws{l}{p}")
                    nc.sync.dma_start(out=t2, in_=ws_in[l][p])
                    wsing[(l, p)] = t2

            def finish_layer(l, y):
                """BN + ReLU + mask + build padded f32r slab with shifted copy."""
                yv = y.rearrange("p (r c) -> p r c", c=WP)
                regs = [yv[:, r, 1:257] for r in range(4, 36)]
                sbs, sbh = bn_finish(l, 64, regs, bnps, cb)
                h = hp.tile([128, HFREE], BF16, tag="h")
                nc.scalar.activation(h[0:64, LEAD + WP:LEAD + WP + YFREE], y,
                                     AF.Relu, bias=sbh, scale=sbs)
                hv = h[0:64, LEAD + WP:LEAD + WP + YFREE].rearrange(
                    "p (r c) -> p r c", c=WP)
                nc.vector.tensor_mul(hv[:, 0:4, :], hv[:, 0:4, :], mv_[:, 0:4, :])
                nc.vector.tensor_mul(hv[:, 36:40, :], hv[:, 36:40, :], mv_[:, 4:8, :])
                hcv = h[0:64, LEAD + WP:LEAD + WP + YFREE].rearrange(
                    "p (r c) -> p c r", c=WP)
                nc.vector.memset(hcv[:, 0, :], 0.0)
                nc.vector.memset(hcv[:, 257, :], 0.0)
                nc.vector.memset(h[0:64, 0:LEAD + WP], 0.0)
                nc.vector.memset(h[0:64, LEAD + WP + YFREE:HFREE], 0.0)
                nc.vector.tensor_copy(h[64:128, 0:HFREE - WP],
                                      h[0:64, WP:HFREE])
                nc.vector.memset(h[64:128, HFREE - WP:HFREE], 0.0)
                return h

            # conv0 (im2col input, K=27, one stream)
            with tc.tile_pool(name="x0p", bufs=1) as x0p:
                x0t = x0p.tile([27, YFREE], BF16)
                nc.sync.dma_start(out=x0t, in_=x0)
                y = cb.tile([64, YFREE], F32, tag="y")
                for (s, L) in TILES:
                    ps = cps.tile([64, 512], F32, tag="cps")
                    nc.tensor.matmul(ps[:, 0:L], w0, x0t[:, s:s + L],
                                     start=True, stop=True)
                    nc.scalar.copy(y[:, s:s + L], ps[:, 0:L])
                h = finish_layer(0, y)

            # conv1, conv2 (6 streams: 3 pairs K=128 + 3 singles K=64)
            GROUP = 6
            for l in (1, 2):
                y = cb.tile([64, YFREE], F32, tag="y")
                for g0 in range(0, len(TILES), GROUP):
                    grp = TILES[g0:g0 + GROUP]
                    pss = [cps.tile([64, 512], F32, tag="cps", name=f"cps{g0}_{i}")
                           for i in range(len(grp))]
                    for p in range(3):
                        for ps, (s, L) in zip(pss, grp):
                            o = LEAD + 516 + s + p - 1
                            nc.tensor.matmul(ps[:, 0:L], wsing[(l, p)],
                                             h[0:64, o:o + L],
                                             start=(p == 0), stop=False)
                    for p in range(3):
                        for ps, (s, L) in zip(pss, grp):
                            o = LEAD + s + p - 1
                            nc.tensor.matmul(ps[:, 0:L], wpair[(l, p)],
                                             h[0:128, o:o + L],
                                             start=False, stop=(p == 2))
                    for ps, (s, L) in zip(pss, grp):
                        nc.scalar.copy(y[:, s:s + L], ps[:, 0:L])
                h = finish_layer(l, y)

            # conv3: output streamed in patch order (gy, py, px, gx)
            def c3rhs(p0, np_, off):
                wide = h[p0:p0 + np_, off:off + 2 * WP]
                w2 = wide.rearrange("p (py c) -> p py c", py=2)
                w3 = w2[:, :, 0:256]
                return w3.rearrange("p py (gx px) -> p py px gx", px=4)

            y3 = cb.tile([2, 8192], F32, tag="y")
            T3 = [(gy, ph) for gy in range(8) for ph in range(2)]
            for g0 in range(0, 16, 6):
                grp = T3[g0:g0 + 6]
                pss = [cps.tile([64, 512], F32, tag="cps", name=f"cps{g0}_{i}")
                           for i in range(len(grp))]
                bases = [LEAD + (5 + 4 * gy + 2 * ph) * WP + 1 for gy, ph in grp]
                for p in range(3):
                    for ps, base in zip(pss, bases):
                        nc.tensor.matmul(ps[0:2, :], wsing[(3, p)],
                                         c3rhs(0, 64, base + WP + (p - 1)),
                                         start=(p == 0), stop=False)
                for p in range(3):
                    for ps, base in zip(pss, bases):
                        nc.tensor.matmul(ps[0:2, :], wpair[(3, p)],
                                         c3rhs(0, 128, base + (p - 1) - WP),
                                         start=False, stop=(p == 2))
                for ps, (gy, ph) in zip(pss, grp):
                    t = gy * 2 + ph
                    nc.scalar.copy(y3[:, t * 512:(t + 1) * 512], ps[0:2, :])
            regs3 = [y3[:, i * 512:(i + 1) * 512] for i in range(16)]
            sbs3, sbh3 = bn_finish(3, 2, regs3, bnps, cb)
            nc.scalar.activation(y3, y3, AF.Relu, bias=sbh3, scale=sbs3)

            # scatter y3 -> patch-major DRAM [16(k=py*4+px), 1024(c,gy,gx)]
            y3p = dr.tile([16, 1024], F32, tag="y3p")
            y5 = y3.rearrange("p (gy py px gx) -> p gy py px gx",
                              gy=8, py=4, px=4)
            y3pr = y3p.rearrange("k (c gy gx) -> k c gy gx", c=2, gy=8)
            for py in range(4):
                for px in range(4):
                    nc.sync.dma_start(out=y3pr[py * 4 + px],
                                      in_=y5[:, :, py, px, :])

        # ---------------- patch augment + AllGather ----------------
        agin = dr.tile([18, 1024], F32, tag="agin")
        gath = dr.tile([8, 18, 1024], F32, tag="gath")
        with tc.tile_pool(name="db", bufs=1) as db, \
             tc.tile_pool(name="sqps", bufs=2, space="PSUM") as sqps:
            Praw = db.tile([16, 1024], F32)
            nc.sync.dma_start(out=Praw, in_=y3p)
            Q = db.tile([16, 1024], F32)
            nc.vector.tensor_mul(Q, Praw, Praw)
            ones16 = db.tile([16, 1], F32)
            nc.vector.memset(ones16, 1.0)
            sqv = db.tile([1, 1024], F32)
            for j in range(2):
                pq = sqps.tile([1, 512], F32, tag="pq")
                nc.tensor.matmul(pq, ones16, Q[:, j * 512:(j + 1) * 512],
                                 start=True, stop=True)
                nc.scalar.copy(sqv[:, j * 512:(j + 1) * 512], pq)
            B16 = db.tile([16, 1024], F32)
            nc.vector.tensor_scalar_mul(B16, Praw, -2.0)
            ones1k = db.tile([1, 1024], F32)
            nc.vector.memset(ones1k, 1.0)
            nc.sync.dma_start(out=agin[0:16, :], in_=B16)
            nc.sync.dma_start(out=agin[16:17, :], in_=ones1k)
            nc.sync.dma_start(out=agin[17:18, :], in_=sqv)
            nc.gpsimd.collective_compute(
                "AllGather", ALU.bypass,
                replica_groups=[list(range(NCORES))],
                ins=[agin.opt()], outs=[gath.opt()])

        # ---------------- distance phase ----------------
        with tc.tile_pool(name="dist", bufs=1) as dist, \
             tc.tile_pool(name="stg", bufs=2) as stg, \
             tc.tile_pool(name="dps", bufs=8, space="PSUM") as dps:
            lhsT = dist.tile([128, 1024], F32)
            nc.sync.dma_start(out=lhsT[0:16, :], in_=agin[0:16, :])
            nc.sync.dma_start(out=lhsT[16:17, :], in_=agin[17:18, :])
            nc.sync.dma_start(out=lhsT[17:18, :], in_=agin[16:17, :])
            nc.vector.tensor_scalar_mul(lhsT[0:16, :], lhsT[0:16, :], -0.5)
            rhs = dist.tile([128, 8192], F32)
            for j in range(16):
                c, k = j // 8, j % 8
                nc.sync.dma_start(out=rhs[0:18, j * 512:(j + 1) * 512],
                                  in_=gath[k, :, c * 512:(c + 1) * 512])
            # replicate the 18 aug rows into 4 PE row-group strips so 4
            # K=18 matmuls run concurrently (tile_position row packing)
            for b in (32, 64, 96):
                nc.vector.tensor_copy(lhsT[b:b + 18, :], lhsT[0:18, :])
                nc.vector.tensor_copy(rhs[b:b + 18, :], rhs[0:18, :])
            for m in range(8):
                stage = stg.tile([128, 8192], F32, tag="stage")
                for n in range(16):
                    b = 32 * ((m * 16 + n) % 4)
                    ps = dps.tile([128, 512], F32, tag="dp")
                    nc.tensor.matmul(ps, lhsT[b:b + 18, m * 128:(m + 1) * 128],
                                     rhs[b:b + 18, n * 512:(n + 1) * 512],
                                     start=True, stop=True,
                                     tile_position=(b, 0))
                    nc.vector.tensor_scalar_max(stage[:, n * 512:(n + 1) * 512],
                                                ps, 0.0)
                    nc.scalar.activation(stage[:, n * 512:(n + 1) * 512],
                                         stage[:, n * 512:(n + 1) * 512], AF.Sqrt)
                nc.sync.dma_start(out=out[m * 128:(m + 1) * 128, :], in_=stage)
    nc.finalize()
    return nc


def _prep_inputs(x, ws_, gs, bes):
    """Per-core numpy input dicts."""
    BF = ml_dtypes.bfloat16
    xp = np.pad(x[0], ((0, 0), (5, 5), (2, 3))).astype(np.float32)
    w0 = ws_[0]
    w0T = np.ascontiguousarray(
        w0.transpose(2, 3, 1, 0).reshape(27, 64)).astype(BF)
    wp, wsg = {}, {}
    for l in (1, 2, 3):
        w = ws_[l]
        wp[l] = np.ascontiguousarray(np.stack(
            [np.concatenate([w[:, :, 0, p].T, w[:, :, 1, p].T], 0)
             for p in range(3)])).astype(BF)
        wsg[l] = np.ascontiguousarray(np.stack(
            [w[:, :, 2, p].T for p in range(3)])).astype(BF)
    g_all = np.concatenate([np.asarray(g, np.float32).ravel() for g in gs]
                           ).reshape(1, 194)
    be_all = np.concatenate([np.asarray(b, np.float32).ravel() for b in bes]
                            ).reshape(1, 194)
    in_maps = []
    for k in range(NCORES):
        col = np.empty((27, ROWS, WP), np.float32)
        for dy in range(3):
            for dx in range(3):
                for ci in range(3):
                    r0 = 32 * k + dy
                    col[(dy * 3 + dx) * 3 + ci] = xp[ci, r0:r0 + ROWS, dx:dx + WP]
        mask = np.zeros((8, WP), np.float32)
        for i, r in enumerate([0, 1, 2, 3, 36, 37, 38, 39]):
            ir = 32 * k - 4 + r
            if 0 <= ir < 256:
                mask[i, 1:257] = 1.0
        in_maps.append(dict(
            x0=np.ascontiguousarray(col.reshape(27, YFREE)).astype(BF),
            w0T=w0T, wp1=wp[1], ws1=wsg[1], wp2=wp[2], ws2=wsg[2],
            wp3=wp[3], ws3=wsg[3], g_all=g_all, be_all=be_all,
            mask8=np.ascontiguousarray(mask.reshape(1, 8 * WP)).astype(BF)))
    return in_maps


def kernel(x, w0, b0, g0, be0, w1, b1, g1, be1, w2, b2, g2, be2,
           w3, b3, g3, be3):
    # conv bias b_i cancels exactly inside BatchNorm (mean absorbs it); unused.
    if "nc" not in _CACHE:
        _CACHE["nc"] = build()
    nc = _CACHE["nc"]
    in_maps = _prep_inputs(
        np.asarray(x, np.float32),
        [np.asarray(w, np.float32) for w in (w0, w1, w2, w3)],
        (g0, g1, g2, g3), (be0, be1, be2, be3))
    res = run_bass_kernel_spmd(nc, in_maps, list(range(NCORES)))
    D = np.empty((8192, 8192), np.float32)
    for k in range(NCORES):
        o = res.results[k]["out"]
        for c in range(2):
            D[c * 4096 + k * 512: c * 4096 + (k + 1) * 512, :] = \
                o[c * 512:(c + 1) * 512, :]
    return D


# revision 20
# speedup vs baseline: 1.0629x; 1.0629x over previous
"""TRN2 Bass kernel for nn_KNN_model (conv stack + pairwise patch distances).

Strategy (8 NeuronCores, SPMD):
  - Convs sharded over H: each core computes a 40-row slab (32 owned + 4 halo
    each side) through all 4 conv+BN+ReLU layers in float32r (TF32-like) on PE.
    3x3 conv = 6 matmul streams per tile: 3 K=128 pairs (top+mid tap rows via a
    partition-shifted slab copy) + 3 K=64 singles (bottom tap row).
  - BN stats: per-core partial (mean, var) via bn_stats/bn_aggr over owned rows,
    tiny AllGather + PE ones-matmul reduce, scale/shift transposed to
    per-partition vectors via K=1 matmuls; applied fused in one ACT pass
    (relu(scale*y+shift)) that also casts to f32r for the next conv.
  - Out-of-image halo rows are zeroed with a per-core mask input (SPMD-safe).
  - Final features -> patch matrix (16 x 1024 local) via a strided scatter DMA,
    augmented to 18 rows (p, sq, ones), AllGathered; distance block
    D[i,j] = sqrt(relu(sq_i + sq_j - 2 p_i.p_j)) computed as ONE fp32 K=18
    matmul per [128,512] tile, DVE relu, ACT sqrt, 4MB DMAs out.
"""
import numpy as np
import ml_dtypes
import concourse.bacc as bacc
import concourse.bass as bass
import concourse.tile as tile
from concourse import mybir
from concourse.bass_utils import run_bass_kernel_spmd

F32 = mybir.dt.float32
F32R = mybir.dt.float32r
BF16 = mybir.dt.bfloat16
AF = mybir.ActivationFunctionType
ALU = mybir.AluOpType

NCORES = 8
WP = 258            # padded row width (256 + 2 pad cols)
ROWS = 40           # ext slab rows per core (32 owned + 4 halo each side)
LEAD = 4            # lead margin so tap offsets never go negative
HROWS = 42          # slab rows + 1 pad row top/bottom
HFREE = LEAD + HROWS * WP + 4   # 10844
YFREE = ROWS * WP   # 10320
EPS = 1e-5
GOFF = [0, 64, 128, 192]        # g/be packing offsets per layer
COUT = [64, 64, 64, 2]

_CACHE = {}


def _conv_tiles():
    out, s = [], 0
    while s < YFREE:
        L = min(512, YFREE - s)
        out.append((s, L))
        s += L
    return out


def build():
    nc = bacc.Bacc(trn_type="TRN2", num_devices=NCORES)
    x0 = nc.dram_tensor("x0", [27, YFREE], F32, kind="ExternalInput").ap()
    w0T = nc.dram_tensor("w0T", [27, 64], F32, kind="ExternalInput").ap()
    wp_in, ws_in = {}, {}
    for l in (1, 2, 3):
        co = COUT[l]
        wp_in[l] = nc.dram_tensor(f"wp{l}", [3, 128, co], F32, kind="ExternalInput").ap()
        ws_in[l] = nc.dram_tensor(f"ws{l}", [3, 64, co], F32, kind="ExternalInput").ap()
    g_all = nc.dram_tensor("g_all", [1, 194], F32, kind="ExternalInput").ap()
    be_all = nc.dram_tensor("be_all", [1, 194], F32, kind="ExternalInput").ap()
    mask8 = nc.dram_tensor("mask8", [1, 8 * WP], F32, kind="ExternalInput").ap()
    out = nc.dram_tensor("out", [1024, 8192], F32, kind="ExternalOutput").ap()

    TILES = _conv_tiles()

    with tile.TileContext(nc) as tc:
      with tc.tile_pool(name="pers", bufs=1) as pers, \
           tc.tile_pool(name="dr", bufs=1, space="DRAM") as dr:
        gsb = pers.tile([1, 194], F32)
        nc.sync.dma_start(out=gsb, in_=g_all)
        besb = pers.tile([1, 194], F32)
        nc.sync.dma_start(out=besb, in_=be_all)
        ones1 = pers.tile([1, 1], F32)
        nc.vector.memset(ones1, 1.0)
        ones8 = pers.tile([8, 1], F32)
        nc.vector.memset(ones8, 0.125)   # 1/8 for mean-of-cores matmul
        epst = pers.tile([1, 1], F32)
        nc.vector.memset(epst, EPS)

        def bn_finish(l, C, regions, bnps, sbp):
            """Cross-core BN: partial stats -> AllGather -> scale/shift [C,1]."""
            # each region is [C, k, <=512]; bn_stats keeps non-innermost dims
            n = sum(r.shape[1] if r.ndim == 3 else 1 for r in regions)
            st = sbp.tile([C, n, 6], F32, tag=f"st{l}")
            i = 0
            for ap in regions:
                k = ap.shape[1] if ap.ndim == 3 else 1
                o = st[:, i:i + k, :] if ap.ndim == 3 else st[:, i, :]
                nc.vector.bn_stats(out=o, in_=ap)
                i += k
            mvt = sbp.tile([C, 2], F32, tag=f"mv{l}")
            nc.vector.bn_aggr(out=mvt, in_=st)
            sti = dr.tile([C, 2], F32, tag=f"sti{l}")
            sto = dr.tile([NCORES, C, 2], F32, tag=f"sto{l}")
            nc.gpsimd.dma_start(out=sti, in_=mvt)
            nc.gpsimd.collective_compute(
                "AllGather", ALU.bypass,
                replica_groups=[list(range(NCORES))],
                ins=[sti.opt()], outs=[sto.opt()])
            G = sbp.tile([8, 2 * C], F32, tag=f"G{l}")
            nc.sync.dma_start(out=G, in_=sto.rearrange("k c two -> k (c two)"))
            Gv = G.rearrange("p (c two) -> p c two", two=2)
            m2 = sbp.tile([8, C], F32, tag=f"m2{l}")
            nc.vector.tensor_mul(m2, Gv[:, :, 0], Gv[:, :, 0])
            pavg = bnps.tile([1, 2 * C], F32, tag="bn")
            nc.tensor.matmul(pavg, ones8, G, start=True, stop=True)
            pavg2 = bnps.tile([1, C], F32, tag="bn")
            nc.tensor.matmul(pavg2, ones8, m2, start=True, stop=True)
            A1 = sbp.tile([1, 2 * C], F32, tag=f"A1{l}")
            nc.scalar.copy(A1, pavg)
            A2 = sbp.tile([1, C], F32, tag=f"A2{l}")
            nc.scalar.copy(A2, pavg2)
            A1v = A1.rearrange("p (c two) -> p c two", two=2)
            am, av = A1v[:, :, 0], A1v[:, :, 1]
            t1 = sbp.tile([1, C], F32, tag=f"t1{l}")
            nc.vector.tensor_mul(t1, am, am)       # E[m]^2
            t2 = sbp.tile([1, C], F32, tag=f"t2{l}")
            nc.vector.tensor_sub(t2, A2, t1)       # Var(means)
            t3 = sbp.tile([1, C], F32, tag=f"t3{l}")
            nc.vector.tensor_add(t3, t2, av)       # + E[var] = total var
            sd = sbp.tile([1, C], F32, tag=f"sd{l}")
            nc.scalar.activation(sd, t3, AF.Sqrt, bias=epst)
            rs = sbp.tile([1, C], F32, tag=f"rs{l}")
            nc.vector.reciprocal(rs, sd)
            off = GOFF[l]
            scl = sbp.tile([1, C], F32, tag=f"scl{l}")
            nc.vector.tensor_mul(scl, gsb[:, off:off + C], rs)
            sh0 = sbp.tile([1, C], F32, tag=f"sh0{l}")
            nc.vector.tensor_mul(sh0, am, scl)
            sh = sbp.tile([1, C], F32, tag=f"sh{l}")
            nc.vector.tensor_sub(sh, besb[:, off:off + C], sh0)
            psc = bnps.tile([C, 1], F32, tag="bn")
            nc.tensor.matmul(psc, scl, ones1, start=True, stop=True)
            psh = bnps.tile([C, 1], F32, tag="bn")
            nc.tensor.matmul(psh, sh, ones1, start=True, stop=True)
            sbs = sbp.tile([C, 1], F32, tag=f"sbs{l}")
            nc.scalar.copy(sbs, psc)
            sbh = sbp.tile([C, 1], F32, tag=f"sbh{l}")
            nc.scalar.copy(sbh, psh)
            return sbs, sbh

        # ---------------- conv phase ----------------
        with tc.tile_pool(name="cb", bufs=1) as cb, \
             tc.tile_pool(name="hp", bufs=2) as hp, \
             tc.tile_pool(name="cps", bufs=6, space="PSUM") as cps, \
             tc.tile_pool(name="bnps", bufs=2, space="PSUM") as bnps:
            mskf = cb.tile([64, 8 * WP], F32)
            nc.gpsimd.dma_start(out=mskf, in_=mask8.partition_broadcast(64))
            mv_ = mskf.rearrange("p (r c) -> p r c", c=WP)
            w0 = cb.tile([27, 64], F32R)
            nc.gpsimd.dma_start(out=w0, in_=w0T)
            wpair, wsing = {}, {}
            for l in (1, 2, 3):
                co = COUT[l]
                for p in range(3):
                    t = cb.tile([128, co], F32R, tag=f"twp{l}{p}")
                    nc.gpsimd.dma_start(out=t, in_=wp_in[l][p])
                    wpair[(l, p)] = t
                    t2 = cb.tile([64, co], F32R, tag=f"tws{l}{p}")
                    nc.gpsimd.dma_start(out=t2, in_=ws_in[l][p])
                    wsing[(l, p)] = t2

            def finish_layer(l, y):
                """BN + ReLU + mask + build padded f32r slab with shifted copy."""
                yv = y.rearrange("p (r c) -> p r c", c=WP)
                regs = [yv[:, r, 1:257] for r in range(4, 36)]
                sbs, sbh = bn_finish(l, 64, regs, bnps, cb)
                h = hp.tile([128, HFREE], F32R, tag="h")
                nc.scalar.activation(h[0:64, LEAD + WP:LEAD + WP + YFREE], y,
                                     AF.Relu, bias=sbh, scale=sbs)
                hv = h[0:64, LEAD + WP:LEAD + WP + YFREE].rearrange(
                    "p (r c) -> p r c", c=WP)
                nc.vector.tensor_mul(hv[:, 0:4, :], hv[:, 0:4, :], mv_[:, 0:4, :])
                nc.vector.tensor_mul(hv[:, 36:40, :], hv[:, 36:40, :], mv_[:, 4:8, :])
                hcv = h[0:64, LEAD + WP:LEAD + WP + YFREE].rearrange(
                    "p (r c) -> p c r", c=WP)
                nc.vector.memset(hcv[:, 0, :].bitcast(F32), 0.0)
                nc.vector.memset(hcv[:, 257, :].bitcast(F32), 0.0)
                nc.vector.memset(h[0:64, 0:LEAD + WP].bitcast(F32), 0.0)
                nc.vector.memset(h[0:64, LEAD + WP + YFREE:HFREE].bitcast(F32), 0.0)
                nc.vector.tensor_copy(h[64:128, 0:HFREE - WP],
                                      h[0:64, WP:HFREE])
                nc.vector.memset(h[64:128, HFREE - WP:HFREE].bitcast(F32), 0.0)
                return h

            # conv0 (im2col input, K=27, one stream)
            with tc.tile_pool(name="x0p", bufs=1) as x0p:
                x0t = x0p.tile([27, YFREE], F32R)
                nc.gpsimd.dma_start(out=x0t, in_=x0)
                y = cb.tile([64, YFREE], F32, tag="y")
                for (s, L) in TILES:
                    ps = cps.tile([64, 512], F32, tag="cps")
                    nc.tensor.matmul(ps[:, 0:L], w0, x0t[:, s:s + L],
                                     start=True, stop=True)
                    nc.scalar.copy(y[:, s:s + L], ps[:, 0:L])
                h = finish_layer(0, y)

            # conv1, conv2 (6 streams: 3 pairs K=128 + 3 singles K=64)
            GROUP = 6
            for l in (1, 2):
                y = cb.tile([64, YFREE], F32, tag="y")
                for g0 in range(0, len(TILES), GROUP):
                    grp = TILES[g0:g0 + GROUP]
                    pss = [cps.tile([64, 512], F32, tag="cps", name=f"cps{g0}_{i}")
                           for i in range(len(grp))]
                    for p in range(3):
                        for ps, (s, L) in zip(pss, grp):
                            o = LEAD + 516 + s + p - 1
                            nc.tensor.matmul(ps[:, 0:L], wsing[(l, p)],
                                             h[0:64, o:o + L],
                                             start=(p == 0), stop=False)
                    for p in range(3):
                        for ps, (s, L) in zip(pss, grp):
                            o = LEAD + s + p - 1
                            nc.tensor.matmul(ps[:, 0:L], wpair[(l, p)],
                                             h[0:128, o:o + L],
                                             start=False, stop=(p == 2))
                    for ps, (s, L) in zip(pss, grp):
                        nc.scalar.copy(y[:, s:s + L], ps[:, 0:L])
                h = finish_layer(l, y)

            # conv3: output streamed in patch order (gy, py, px, gx)
            def c3rhs(p0, np_, off):
                wide = h[p0:p0 + np_, off:off + 2 * WP]
                w2 = wide.rearrange("p (py c) -> p py c", py=2)
                w3 = w2[:, :, 0:256]
                return w3.rearrange("p py (gx px) -> p py px gx", px=4)

            y3 = cb.tile([2, 8192], F32, tag="y")
            T3 = [(gy, ph) for gy in range(8) for ph in range(2)]
            for g0 in range(0, 16, 6):
                grp = T3[g0:g0 + 6]
                pss = [cps.tile([64, 512], F32, tag="cps", name=f"cps{g0}_{i}")
                           for i in range(len(grp))]
                bases = [LEAD + (5 + 4 * gy + 2 * ph) * WP + 1 for gy, ph in grp]
                for p in range(3):
                    for ps, base in zip(pss, bases):
                        nc.tensor.matmul(ps[0:2, :], wsing[(3, p)],
                                         c3rhs(0, 64, base + WP + (p - 1)),
                                         start=(p == 0), stop=False)
                for p in range(3):
                    for ps, base in zip(pss, bases):
                        nc.tensor.matmul(ps[0:2, :], wpair[(3, p)],
                                         c3rhs(0, 128, base + (p - 1) - WP),
                                         start=False, stop=(p == 2))
                for ps, (gy, ph) in zip(pss, grp):
                    t = gy * 2 + ph
                    nc.scalar.copy(y3[:, t * 512:(t + 1) * 512], ps[0:2, :])
            regs3 = [y3[:, i * 512:(i + 1) * 512] for i in range(16)]
            sbs3, sbh3 = bn_finish(3, 2, regs3, bnps, cb)
            nc.scalar.activation(y3, y3, AF.Relu, bias=sbh3, scale=sbs3)

            # scatter y3 -> patch-major DRAM [16(k=py*4+px), 1024(c,gy,gx)]
            y3p = dr.tile([16, 1024], F32, tag="y3p")
            y5 = y3.rearrange("p (gy py px gx) -> p gy py px gx",
                              gy=8, py=4, px=4)
            y3pr = y3p.rearrange("k (c gy gx) -> k c gy gx", c=2, gy=8)
            for py in range(4):
                for px in range(4):
                    nc.sync.dma_start(out=y3pr[py * 4 + px],
                                      in_=y5[:, :, py, px, :])

        # ---------------- patch augment + AllGather ----------------
        agin = dr.tile([18, 1024], F32, tag="agin")
        gath = dr.tile([8, 18, 1024], F32, tag="gath")
        with tc.tile_pool(name="db", bufs=1) as db, \
             tc.tile_pool(name="sqps", bufs=2, space="PSUM") as sqps:
            Praw = db.tile([16, 1024], F32)
            nc.sync.dma_start(out=Praw, in_=y3p)
            Q = db.tile([16, 1024], F32)
            nc.vector.tensor_mul(Q, Praw, Praw)
            ones16 = db.tile([16, 1], F32)
            nc.vector.memset(ones16, 1.0)
            sqv = db.tile([1, 1024], F32)
            for j in range(2):
                pq = sqps.tile([1, 512], F32, tag="pq")
                nc.tensor.matmul(pq, ones16, Q[:, j * 512:(j + 1) * 512],
                                 start=True, stop=True)
                nc.scalar.copy(sqv[:, j * 512:(j + 1) * 512], pq)
            B16 = db.tile([16, 1024], F32)
            nc.vector.tensor_scalar_mul(B16, Praw, -2.0)
            ones1k = db.tile([1, 1024], F32)
            nc.vector.memset(ones1k, 1.0)
            nc.sync.dma_start(out=agin[0:16, :], in_=B16)
            nc.sync.dma_start(out=agin[16:17, :], in_=ones1k)
            nc.sync.dma_start(out=agin[17:18, :], in_=sqv)
            nc.gpsimd.collective_compute(
                "AllGather", ALU.bypass,
                replica_groups=[list(range(NCORES))],
                ins=[agin.opt()], outs=[gath.opt()])

        # ---------------- distance phase ----------------
        with tc.tile_pool(name="dist", bufs=1) as dist, \
             tc.tile_pool(name="stg", bufs=2) as stg, \
             tc.tile_pool(name="dps", bufs=8, space="PSUM") as dps:
            lhsT = dist.tile([128, 1024], F32)
            nc.sync.dma_start(out=lhsT[0:16, :], in_=agin[0:16, :])
            nc.sync.dma_start(out=lhsT[16:17, :], in_=agin[17:18, :])
            nc.sync.dma_start(out=lhsT[17:18, :], in_=agin[16:17, :])
            nc.vector.tensor_scalar_mul(lhsT[0:16, :], lhsT[0:16, :], -0.5)
            rhs = dist.tile([128, 8192], F32)
            for j in range(16):
                c, k = j // 8, j % 8
                nc.sync.dma_start(out=rhs[0:18, j * 512:(j + 1) * 512],
                                  in_=gath[k, :, c * 512:(c + 1) * 512])
            # replicate the 18 aug rows into 4 PE row-group strips so 4
            # K=18 matmuls run concurrently (tile_position row packing)
            for b in (32, 64, 96):
                nc.vector.tensor_copy(lhsT[b:b + 18, :], lhsT[0:18, :])
                nc.vector.tensor_copy(rhs[b:b + 18, :], rhs[0:18, :])
            for m in range(8):
                stage = stg.tile([128, 8192], F32, tag="stage")
                for n in range(16):
                    b = 32 * ((m * 16 + n) % 4)
                    ps = dps.tile([128, 512], F32, tag="dp")
                    nc.tensor.matmul(ps, lhsT[b:b + 18, m * 128:(m + 1) * 128],
                                     rhs[b:b + 18, n * 512:(n + 1) * 512],
                                     start=True, stop=True,
                                     tile_position=(b, 0))
                    nc.vector.tensor_scalar_max(stage[:, n * 512:(n + 1) * 512],
                                                ps, 0.0)
                    nc.scalar.activation(stage[:, n * 512:(n + 1) * 512],
                                         stage[:, n * 512:(n + 1) * 512], AF.Sqrt)
                nc.sync.dma_start(out=out[m * 128:(m + 1) * 128, :], in_=stage)
    nc.finalize()
    return nc


def _prep_inputs(x, ws_, gs, bes):
    """Per-core numpy input dicts."""
    BF = ml_dtypes.bfloat16
    xp = np.pad(x[0], ((0, 0), (5, 5), (2, 3))).astype(np.float32)
    w0 = ws_[0]
    w0T = np.ascontiguousarray(
        w0.transpose(2, 3, 1, 0).reshape(27, 64)).astype(np.float32)
    wp, wsg = {}, {}
    for l in (1, 2, 3):
        w = ws_[l]
        wp[l] = np.ascontiguousarray(np.stack(
            [np.concatenate([w[:, :, 0, p].T, w[:, :, 1, p].T], 0)
             for p in range(3)])).astype(np.float32)
        wsg[l] = np.ascontiguousarray(np.stack(
            [w[:, :, 2, p].T for p in range(3)])).astype(np.float32)
    g_all = np.concatenate([np.asarray(g, np.float32).ravel() for g in gs]
                           ).reshape(1, 194)
    be_all = np.concatenate([np.asarray(b, np.float32).ravel() for b in bes]
                            ).reshape(1, 194)
    in_maps = []
    for k in range(NCORES):
        col = np.empty((27, ROWS, WP), np.float32)
        for dy in range(3):
            for dx in range(3):
                for ci in range(3):
                    r0 = 32 * k + dy
                    col[(dy * 3 + dx) * 3 + ci] = xp[ci, r0:r0 + ROWS, dx:dx + WP]
        mask = np.zeros((8, WP), np.float32)
        for i, r in enumerate([0, 1, 2, 3, 36, 37, 38, 39]):
            ir = 32 * k - 4 + r
            if 0 <= ir < 256:
                mask[i, 1:257] = 1.0
        in_maps.append(dict(
            x0=np.ascontiguousarray(col.reshape(27, YFREE)),
            w0T=w0T, wp1=wp[1], ws1=wsg[1], wp2=wp[2], ws2=wsg[2],
            wp3=wp[3], ws3=wsg[3], g_all=g_all, be_all=be_all,
            mask8=np.ascontiguousarray(mask.reshape(1, 8 * WP))))
    return in_maps


def kernel(x, w0, b0, g0, be0, w1, b1, g1, be1, w2, b2, g2, be2,
           w3, b3, g3, be3):
    # conv bias b_i cancels exactly inside BatchNorm (mean absorbs it); unused.
    if "nc" not in _CACHE:
        _CACHE["nc"] = build()
    nc = _CACHE["nc"]
    in_maps = _prep_inputs(
        np.asarray(x, np.float32),
        [np.asarray(w, np.float32) for w in (w0, w1, w2, w3)],
        (g0, g1, g2, g3), (be0, be1, be2, be3))
    res = run_bass_kernel_spmd(nc, in_maps, list(range(NCORES)))
    D = np.empty((8192, 8192), np.float32)
    for k in range(NCORES):
        o = res.results[k]["out"]
        for c in range(2):
            D[c * 4096 + k * 512: c * 4096 + (k + 1) * 512, :] = \
                o[c * 512:(c + 1) * 512, :]
    return D


# revision 21
# speedup vs baseline: 22363.3420x; 21040.7311x over previous
"""TRN2 Bass kernel for nn_KNN_model (conv stack + pairwise patch distances).

Strategy (8 NeuronCores, SPMD):
  - Convs sharded over H: each core computes a 40-row slab (32 owned + 4 halo
    each side) through all 4 conv+BN+ReLU layers in float32r (TF32-like) on PE.
    3x3 conv = 6 matmul streams per tile: 3 K=128 pairs (top+mid tap rows via a
    partition-shifted slab copy) + 3 K=64 singles (bottom tap row).
  - BN stats: per-core partial (mean, var) via bn_stats/bn_aggr over owned rows,
    tiny AllGather + PE ones-matmul reduce, scale/shift transposed to
    per-partition vectors via K=1 matmuls; applied fused in one ACT pass
    (relu(scale*y+shift)) that also casts to f32r for the next conv.
  - Out-of-image halo rows are zeroed with a per-core mask input (SPMD-safe).
  - Final features -> patch matrix (16 x 1024 local) via a strided scatter DMA,
    augmented to 18 rows (p, sq, ones), AllGathered; distance block
    D[i,j] = sqrt(relu(sq_i + sq_j - 2 p_i.p_j)) computed as ONE fp32 K=18
    matmul per [128,512] tile, DVE relu, ACT sqrt, 4MB DMAs out.
"""
import numpy as np
import ml_dtypes
import concourse.bacc as bacc
import concourse.bass as bass
import concourse.tile as tile
from concourse import mybir
from concourse.bass_utils import run_bass_kernel_spmd

F32 = mybir.dt.float32
F32R = mybir.dt.float32r
BF16 = mybir.dt.bfloat16
AF = mybir.ActivationFunctionType
ALU = mybir.AluOpType

NCORES = 8
WP = 258            # padded row width (256 + 2 pad cols)
ROWS = 40           # ext slab rows per core (32 owned + 4 halo each side)
LEAD = 4            # lead margin so tap offsets never go negative
HROWS = 42          # slab rows + 1 pad row top/bottom
HFREE = LEAD + HROWS * WP + 4   # 10844
YFREE = ROWS * WP   # 10320
EPS = 1e-5
GOFF = [0, 64, 128, 192]        # g/be packing offsets per layer
COUT = [64, 64, 64, 2]

_CACHE = {}


def _conv_tiles(s0=0, s1=YFREE):
    out, s = [], s0
    while s < s1:
        L = min(512, s1 - s)
        out.append((s, L))
        s += L
    return out


def build():
    nc = bacc.Bacc(trn_type="TRN2", num_devices=NCORES)
    x0 = nc.dram_tensor("x0", [27, YFREE], F32, kind="ExternalInput").ap()
    w0T = nc.dram_tensor("w0T", [27, 64], F32, kind="ExternalInput").ap()
    wp_in, ws_in = {}, {}
    for l in (1, 2, 3):
        co = COUT[l]
        wp_in[l] = nc.dram_tensor(f"wp{l}", [3, 128, co], F32, kind="ExternalInput").ap()
        ws_in[l] = nc.dram_tensor(f"ws{l}", [3, 64, co], F32, kind="ExternalInput").ap()
    g_all = nc.dram_tensor("g_all", [1, 194], F32, kind="ExternalInput").ap()
    be_all = nc.dram_tensor("be_all", [1, 194], F32, kind="ExternalInput").ap()
    mask8 = nc.dram_tensor("mask8", [1, 8 * WP], F32, kind="ExternalInput").ap()
    out = nc.dram_tensor("out", [1024, 8192], F32, kind="ExternalOutput").ap()

    TILES = {0: _conv_tiles(WP, 39 * WP),
             1: _conv_tiles(2 * WP, 38 * WP),
             2: _conv_tiles(3 * WP, 37 * WP)}

    with tile.TileContext(nc) as tc:
      with tc.tile_pool(name="pers", bufs=1) as pers, \
           tc.tile_pool(name="dr", bufs=1, space="DRAM") as dr:
        gsb = pers.tile([1, 194], F32)
        nc.sync.dma_start(out=gsb, in_=g_all)
        besb = pers.tile([1, 194], F32)
        nc.sync.dma_start(out=besb, in_=be_all)
        ones1 = pers.tile([1, 1], F32)
        nc.vector.memset(ones1, 1.0)
        ones8 = pers.tile([8, 1], F32)
        nc.vector.memset(ones8, 0.125)   # 1/8 for mean-of-cores matmul
        epst = pers.tile([1, 1], F32)
        nc.vector.memset(epst, EPS)

        def bn_finish(l, C, regions, bnps, sbp):
            """Cross-core BN: partial stats -> AllGather -> scale/shift [C,1]."""
            # each region is [C, k, <=512]; bn_stats keeps non-innermost dims
            n = sum(r.shape[1] if r.ndim == 3 else 1 for r in regions)
            st = sbp.tile([C, n, 6], F32, tag=f"st{l}")
            i = 0
            for ap in regions:
                k = ap.shape[1] if ap.ndim == 3 else 1
                o = st[:, i:i + k, :] if ap.ndim == 3 else st[:, i, :]
                nc.vector.bn_stats(out=o, in_=ap)
                i += k
            mvt = sbp.tile([C, 2], F32, tag=f"mv{l}")
            nc.vector.bn_aggr(out=mvt, in_=st)
            sti = dr.tile([C, 2], F32, tag=f"sti{l}")
            sto = dr.tile([NCORES, C, 2], F32, tag=f"sto{l}")
            nc.gpsimd.dma_start(out=sti, in_=mvt)
            nc.gpsimd.collective_compute(
                "AllGather", ALU.bypass,
                replica_groups=[list(range(NCORES))],
                ins=[sti.opt()], outs=[sto.opt()])
            G = sbp.tile([8, 2 * C], F32, tag=f"G{l}")
            nc.sync.dma_start(out=G, in_=sto.rearrange("k c two -> k (c two)"))
            Gv = G.rearrange("p (c two) -> p c two", two=2)
            m2 = sbp.tile([8, C], F32, tag=f"m2{l}")
            nc.vector.tensor_mul(m2, Gv[:, :, 0], Gv[:, :, 0])
            pavg = bnps.tile([1, 2 * C], F32, tag="bn")
            nc.tensor.matmul(pavg, ones8, G, start=True, stop=True)
            pavg2 = bnps.tile([1, C], F32, tag="bn")
            nc.tensor.matmul(pavg2, ones8, m2, start=True, stop=True)
            A1 = sbp.tile([1, 2 * C], F32, tag=f"A1{l}")
            nc.scalar.copy(A1, pavg)
            A2 = sbp.tile([1, C], F32, tag=f"A2{l}")
            nc.scalar.copy(A2, pavg2)
            A1v = A1.rearrange("p (c two) -> p c two", two=2)
            am, av = A1v[:, :, 0], A1v[:, :, 1]
            t1 = sbp.tile([1, C], F32, tag=f"t1{l}")
            nc.vector.tensor_mul(t1, am, am)       # E[m]^2
            t2 = sbp.tile([1, C], F32, tag=f"t2{l}")
            nc.vector.tensor_sub(t2, A2, t1)       # Var(means)
            t3 = sbp.tile([1, C], F32, tag=f"t3{l}")
            nc.vector.tensor_add(t3, t2, av)       # + E[var] = total var
            sd = sbp.tile([1, C], F32, tag=f"sd{l}")
            nc.scalar.activation(sd, t3, AF.Sqrt, bias=epst)
            rs = sbp.tile([1, C], F32, tag=f"rs{l}")
            nc.vector.reciprocal(rs, sd)
            off = GOFF[l]
            scl = sbp.tile([1, C], F32, tag=f"scl{l}")
            nc.vector.tensor_mul(scl, gsb[:, off:off + C], rs)
            sh0 = sbp.tile([1, C], F32, tag=f"sh0{l}")
            nc.vector.tensor_mul(sh0, am, scl)
            sh = sbp.tile([1, C], F32, tag=f"sh{l}")
            nc.vector.tensor_sub(sh, besb[:, off:off + C], sh0)
            psc = bnps.tile([C, 1], F32, tag="bn")
            nc.tensor.matmul(psc, scl, ones1, start=True, stop=True)
            psh = bnps.tile([C, 1], F32, tag="bn")
            nc.tensor.matmul(psh, sh, ones1, start=True, stop=True)
            sbs = sbp.tile([C, 1], F32, tag=f"sbs{l}")
            nc.scalar.copy(sbs, psc)
            sbh = sbp.tile([C, 1], F32, tag=f"sbh{l}")
            nc.scalar.copy(sbh, psh)
            return sbs, sbh

        # ---------------- conv phase ----------------
        with tc.tile_pool(name="cb", bufs=1) as cb, \
             tc.tile_pool(name="hp", bufs=2) as hp, \
             tc.tile_pool(name="cps", bufs=6, space="PSUM") as cps, \
             tc.tile_pool(name="bnps", bufs=2, space="PSUM") as bnps:
            mskf = cb.tile([64, 8 * WP], F32)
            nc.gpsimd.dma_start(out=mskf, in_=mask8.partition_broadcast(64))
            mv_ = mskf.rearrange("p (r c) -> p r c", c=WP)
            w0 = cb.tile([27, 64], F32R)
            nc.gpsimd.dma_start(out=w0, in_=w0T)
            wpair, wsing = {}, {}
            for l in (1, 2, 3):
                co = COUT[l]
                for p in range(3):
                    t = cb.tile([128, co], F32R, tag=f"twp{l}{p}")
                    nc.gpsimd.dma_start(out=t, in_=wp_in[l][p])
                    wpair[(l, p)] = t
                    t2 = cb.tile([64, co], F32R, tag=f"tws{l}{p}")
                    nc.gpsimd.dma_start(out=t2, in_=ws_in[l][p])
                    wsing[(l, p)] = t2

            def finish_layer(l, y):
                """BN + ReLU + mask + build padded f32r slab with shifted copy."""
                yv = y.rearrange("p (r c) -> p r c", c=WP)
                regs = [yv[:, r, 1:257] for r in range(4, 36)]
                sbs, sbh = bn_finish(l, 64, regs, bnps, cb)
                h = hp.tile([128, HFREE], F32R, tag="h")
                nc.scalar.activation(h[0:64, LEAD + WP:LEAD + WP + YFREE], y,
                                     AF.Relu, bias=sbh, scale=sbs)
                hv = h[0:64, LEAD + WP:LEAD + WP + YFREE].rearrange(
                    "p (r c) -> p r c", c=WP)
                nc.vector.tensor_mul(hv[:, 0:4, :], hv[:, 0:4, :], mv_[:, 0:4, :])
                nc.vector.tensor_mul(hv[:, 36:40, :], hv[:, 36:40, :], mv_[:, 4:8, :])
                hcv = h[0:64, LEAD + WP:LEAD + WP + YFREE].rearrange(
                    "p (r c) -> p c r", c=WP)
                nc.vector.memset(hcv[:, 0, :].bitcast(F32), 0.0)
                nc.vector.memset(hcv[:, 257, :].bitcast(F32), 0.0)
                nc.vector.memset(h[0:64, 0:LEAD + WP].bitcast(F32), 0.0)
                nc.vector.memset(h[0:64, LEAD + WP + YFREE:HFREE].bitcast(F32), 0.0)
                nc.vector.tensor_copy(h[64:128, 0:HFREE - WP],
                                      h[0:64, WP:HFREE])
                nc.vector.memset(h[64:128, HFREE - WP:HFREE].bitcast(F32), 0.0)
                return h

            # conv0 (im2col input, K=27, one stream)
            with tc.tile_pool(name="x0p", bufs=1) as x0p:
                x0t = x0p.tile([27, YFREE], F32R)
                nc.gpsimd.dma_start(out=x0t, in_=x0)
                y = cb.tile([64, YFREE], F32, tag="y")
                for (s, L) in TILES[0]:
                    ps = cps.tile([64, 512], F32, tag="cps")
                    nc.tensor.matmul(ps[:, 0:L], w0, x0t[:, s:s + L],
                                     start=True, stop=True)
                    nc.scalar.copy(y[:, s:s + L], ps[:, 0:L])
                h = finish_layer(0, y)

            # conv1, conv2 (6 streams: 3 pairs K=128 + 3 singles K=64)
            GROUP = 6
            for l in (1, 2):
                y = cb.tile([64, YFREE], F32, tag="y")
                for g0 in range(0, len(TILES[l]), GROUP):
                    grp = TILES[l][g0:g0 + GROUP]
                    pss = [cps.tile([64, 512], F32, tag="cps", name=f"cps{g0}_{i}")
                           for i in range(len(grp))]
                    for p in range(3):
                        for ps, (s, L) in zip(pss, grp):
                            o = LEAD + 516 + s + p - 1
                            nc.tensor.matmul(ps[:, 0:L], wsing[(l, p)],
                                             h[0:64, o:o + L],
                                             start=(p == 0), stop=False)
                    for p in range(3):
                        for ps, (s, L) in zip(pss, grp):
                            o = LEAD + s + p - 1
                            nc.tensor.matmul(ps[:, 0:L], wpair[(l, p)],
                                             h[0:128, o:o + L],
                                             start=False, stop=(p == 2))
                    for ps, (s, L) in zip(pss, grp):
                        nc.scalar.copy(y[:, s:s + L], ps[:, 0:L])
                h = finish_layer(l, y)

            # conv3: output streamed in patch order (gy, py, px, gx)
            def c3rhs(p0, np_, off):
                wide = h[p0:p0 + np_, off:off + 2 * WP]
                w2 = wide.rearrange("p (py c) -> p py c", py=2)
                w3 = w2[:, :, 0:256]
                return w3.rearrange("p py (gx px) -> p py px gx", px=4)

            y3 = cb.tile([2, 8192], F32, tag="y")
            T3 = [(gy, ph) for gy in range(8) for ph in range(2)]
            for g0 in range(0, 16, 6):
                grp = T3[g0:g0 + 6]
                pss = [cps.tile([64, 512], F32, tag="cps", name=f"cps{g0}_{i}")
                           for i in range(len(grp))]
                bases = [LEAD + (5 + 4 * gy + 2 * ph) * WP + 1 for gy, ph in grp]
                for p in range(3):
                    for ps, base in zip(pss, bases):
                        nc.tensor.matmul(ps[0:2, :], wsing[(3, p)],
                                         c3rhs(0, 64, base + WP + (p - 1)),
                                         start=(p == 0), stop=False)
                for p in range(3):
                    for ps, base in zip(pss, bases):
                        nc.tensor.matmul(ps[0:2, :], wpair[(3, p)],
                                         c3rhs(0, 128, base + (p - 1) - WP),
                                         start=False, stop=(p == 2))
                for ps, (gy, ph) in zip(pss, grp):
                    t = gy * 2 + ph
                    nc.scalar.copy(y3[:, t * 512:(t + 1) * 512], ps[0:2, :])
            regs3 = [y3[:, i * 512:(i + 1) * 512] for i in range(16)]
            sbs3, sbh3 = bn_finish(3, 2, regs3, bnps, cb)
            nc.scalar.activation(y3, y3, AF.Relu, bias=sbh3, scale=sbs3)

            # scatter y3 -> patch-major DRAM [16(k=py*4+px), 1024(c,gy,gx)]
            y3p = dr.tile([16, 1024], F32, tag="y3p")
            y5 = y3.rearrange("p (gy py px gx) -> p gy py px gx",
                              gy=8, py=4, px=4)
            y3pr = y3p.rearrange("k (c gy gx) -> k c gy gx", c=2, gy=8)
            for py in range(4):
                for px in range(4):
                    nc.sync.dma_start(out=y3pr[py * 4 + px],
                                      in_=y5[:, :, py, px, :])

        # ---------------- patch augment + AllGather ----------------
        agin = dr.tile([18, 1024], F32, tag="agin")
        gath = dr.tile([8, 18, 1024], F32, tag="gath")
        with tc.tile_pool(name="db", bufs=1) as db, \
             tc.tile_pool(name="sqps", bufs=2, space="PSUM") as sqps:
            Praw = db.tile([16, 1024], F32)
            nc.sync.dma_start(out=Praw, in_=y3p)
            Q = db.tile([16, 1024], F32)
            nc.vector.tensor_mul(Q, Praw, Praw)
            ones16 = db.tile([16, 1], F32)
            nc.vector.memset(ones16, 1.0)
            sqv = db.tile([1, 1024], F32)
            for j in range(2):
                pq = sqps.tile([1, 512], F32, tag="pq")
                nc.tensor.matmul(pq, ones16, Q[:, j * 512:(j + 1) * 512],
                                 start=True, stop=True)
                nc.scalar.copy(sqv[:, j * 512:(j + 1) * 512], pq)
            B16 = db.tile([16, 1024], F32)
            nc.vector.tensor_scalar_mul(B16, Praw, -2.0)
            ones1k = db.tile([1, 1024], F32)
            nc.vector.memset(ones1k, 1.0)
            nc.sync.dma_start(out=agin[0:16, :], in_=B16)
            nc.sync.dma_start(out=agin[16:17, :], in_=ones1k)
            nc.sync.dma_start(out=agin[17:18, :], in_=sqv)
            nc.gpsimd.collective_compute(
                "AllGather", ALU.bypass,
                replica_groups=[list(range(NCORES))],
                ins=[agin.opt()], outs=[gath.opt()])

        # ---------------- distance phase ----------------
        with tc.tile_pool(name="dist", bufs=1) as dist, \
             tc.tile_pool(name="stg", bufs=2) as stg, \
             tc.tile_pool(name="dps", bufs=8, space="PSUM") as dps:
            lhsT = dist.tile([128, 1024], F32)
            nc.sync.dma_start(out=lhsT[0:16, :], in_=agin[0:16, :])
            nc.sync.dma_start(out=lhsT[16:17, :], in_=agin[17:18, :])
            nc.sync.dma_start(out=lhsT[17:18, :], in_=agin[16:17, :])
            nc.vector.tensor_scalar_mul(lhsT[0:16, :], lhsT[0:16, :], -0.5)
            rhs = dist.tile([128, 8192], F32)
            for j in range(16):
                c, k = j // 8, j % 8
                nc.sync.dma_start(out=rhs[0:18, j * 512:(j + 1) * 512],
                                  in_=gath[k, :, c * 512:(c + 1) * 512])
            # replicate the 18 aug rows into 4 PE row-group strips so 4
            # K=18 matmuls run concurrently (tile_position row packing)
            for b in (32, 64, 96):
                nc.vector.tensor_copy(lhsT[b:b + 18, :], lhsT[0:18, :])
                nc.vector.tensor_copy(rhs[b:b + 18, :], rhs[0:18, :])
            for m in range(8):
                stage = stg.tile([128, 8192], F32, tag="stage")
                for n in range(16):
                    b = 32 * ((m * 16 + n) % 4)
                    ps = dps.tile([128, 512], F32, tag="dp")
                    nc.tensor.matmul(ps, lhsT[b:b + 18, m * 128:(m + 1) * 128],
                                     rhs[b:b + 18, n * 512:(n + 1) * 512],
                                     start=True, stop=True,
                                     tile_position=(b, 0))
                    nc.vector.tensor_scalar_max(stage[:, n * 512:(n + 1) * 512],
                                                ps, 0.0)
                    nc.scalar.activation(stage[:, n * 512:(n + 1) * 512],
                                         stage[:, n * 512:(n + 1) * 512], AF.Sqrt)
                nc.sync.dma_start(out=out[m * 128:(m + 1) * 128, :], in_=stage)
    nc.finalize()
    return nc


def _prep_inputs(x, ws_, gs, bes):
    """Per-core numpy input dicts."""
    BF = ml_dtypes.bfloat16
    xp = np.pad(x[0], ((0, 0), (5, 5), (2, 3))).astype(np.float32)
    w0 = ws_[0]
    w0T = np.ascontiguousarray(
        w0.transpose(2, 3, 1, 0).reshape(27, 64)).astype(np.float32)
    wp, wsg = {}, {}
    for l in (1, 2, 3):
        w = ws_[l]
        wp[l] = np.ascontiguousarray(np.stack(
            [np.concatenate([w[:, :, 0, p].T, w[:, :, 1, p].T], 0)
             for p in range(3)])).astype(np.float32)
        wsg[l] = np.ascontiguousarray(np.stack(
            [w[:, :, 2, p].T for p in range(3)])).astype(np.float32)
    g_all = np.concatenate([np.asarray(g, np.float32).ravel() for g in gs]
                           ).reshape(1, 194)
    be_all = np.concatenate([np.asarray(b, np.float32).ravel() for b in bes]
                            ).reshape(1, 194)
    in_maps = []
    for k in range(NCORES):
        col = np.empty((27, ROWS, WP), np.float32)
        for dy in range(3):
            for dx in range(3):
                for ci in range(3):
                    r0 = 32 * k + dy
                    col[(dy * 3 + dx) * 3 + ci] = xp[ci, r0:r0 + ROWS, dx:dx + WP]
        mask = np.zeros((8, WP), np.float32)
        for i, r in enumerate([0, 1, 2, 3, 36, 37, 38, 39]):
            ir = 32 * k - 4 + r
            if 0 <= ir < 256:
                mask[i, 1:257] = 1.0
        in_maps.append(dict(
            x0=np.ascontiguousarray(col.reshape(27, YFREE)),
            w0T=w0T, wp1=wp[1], ws1=wsg[1], wp2=wp[2], ws2=wsg[2],
            wp3=wp[3], ws3=wsg[3], g_all=g_all, be_all=be_all,
            mask8=np.ascontiguousarray(mask.reshape(1, 8 * WP))))
    return in_maps


def kernel(x, w0, b0, g0, be0, w1, b1, g1, be1, w2, b2, g2, be2,
           w3, b3, g3, be3):
    # conv bias b_i cancels exactly inside BatchNorm (mean absorbs it); unused.
    if "nc" not in _CACHE:
        _CACHE["nc"] = build()
    nc = _CACHE["nc"]
    in_maps = _prep_inputs(
        np.asarray(x, np.float32),
        [np.asarray(w, np.float32) for w in (w0, w1, w2, w3)],
        (g0, g1, g2, g3), (be0, be1, be2, be3))
    res = run_bass_kernel_spmd(nc, in_maps, list(range(NCORES)))
    D = np.empty((8192, 8192), np.float32)
    for k in range(NCORES):
        o = res.results[k]["out"]
        for c in range(2):
            D[c * 4096 + k * 512: c * 4096 + (k + 1) * 512, :] = \
                o[c * 512:(c + 1) * 512, :]
    return D


# revision 22
# speedup vs baseline: 23414.0643x; 1.0470x over previous
"""TRN2 Bass kernel for nn_KNN_model (conv stack + pairwise patch distances).

Strategy (8 NeuronCores, SPMD):
  - Convs sharded over H: each core computes a 40-row slab (32 owned + 4 halo
    each side) through all 4 conv+BN+ReLU layers in float32r (TF32-like) on PE.
    3x3 conv = 6 matmul streams per tile: 3 K=128 pairs (top+mid tap rows via a
    partition-shifted slab copy) + 3 K=64 singles (bottom tap row).
  - BN stats: per-core partial (mean, var) via bn_stats/bn_aggr over owned rows,
    tiny AllGather + PE ones-matmul reduce, scale/shift transposed to
    per-partition vectors via K=1 matmuls; applied fused in one ACT pass
    (relu(scale*y+shift)) that also casts to f32r for the next conv.
  - Out-of-image halo rows are zeroed with a per-core mask input (SPMD-safe).
  - Final features -> patch matrix (16 x 1024 local) via a strided scatter DMA,
    augmented to 18 rows (p, sq, ones), AllGathered; distance block
    D[i,j] = sqrt(relu(sq_i + sq_j - 2 p_i.p_j)) computed as ONE fp32 K=18
    matmul per [128,512] tile, DVE relu, ACT sqrt, 4MB DMAs out.
"""
import numpy as np
import ml_dtypes
import concourse.bacc as bacc
import concourse.bass as bass
import concourse.tile as tile
from concourse import mybir
from concourse.bass_utils import run_bass_kernel_spmd

F32 = mybir.dt.float32
F32R = mybir.dt.float32r
BF16 = mybir.dt.bfloat16
AF = mybir.ActivationFunctionType
ALU = mybir.AluOpType

NCORES = 8
WP = 258            # padded row width (256 + 2 pad cols)
ROWS = 40           # ext slab rows per core (32 owned + 4 halo each side)
LEAD = 4            # lead margin so tap offsets never go negative
HROWS = 42          # slab rows + 1 pad row top/bottom
HFREE = LEAD + HROWS * WP + 4   # 10844
YFREE = ROWS * WP   # 10320
EPS = 1e-5
GOFF = [0, 64, 128, 192]        # g/be packing offsets per layer
COUT = [64, 64, 64, 2]

_CACHE = {}


def _conv_tiles(s0=0, s1=YFREE):
    out, s = [], s0
    while s < s1:
        L = min(512, s1 - s)
        out.append((s, L))
        s += L
    return out


def build():
    nc = bacc.Bacc(trn_type="TRN2", num_devices=NCORES)
    x0 = nc.dram_tensor("x0", [27, YFREE], F32, kind="ExternalInput").ap()
    w0T = nc.dram_tensor("w0T", [27, 64], F32, kind="ExternalInput").ap()
    wp_in, ws_in = {}, {}
    for l in (1, 2, 3):
        co = COUT[l]
        wp_in[l] = nc.dram_tensor(f"wp{l}", [3, 128, co], F32, kind="ExternalInput").ap()
        ws_in[l] = nc.dram_tensor(f"ws{l}", [3, 64, co], F32, kind="ExternalInput").ap()
    g_all = nc.dram_tensor("g_all", [1, 194], F32, kind="ExternalInput").ap()
    be_all = nc.dram_tensor("be_all", [1, 194], F32, kind="ExternalInput").ap()
    mask8 = nc.dram_tensor("mask8", [1, 8 * WP], F32, kind="ExternalInput").ap()
    out = nc.dram_tensor("out", [1024, 8192], F32, kind="ExternalOutput").ap()

    TILES = {0: _conv_tiles(WP, 39 * WP),
             1: _conv_tiles(2 * WP, 38 * WP),
             2: _conv_tiles(3 * WP, 37 * WP)}

    with tile.TileContext(nc) as tc:
      with tc.tile_pool(name="pers", bufs=1) as pers, \
           tc.tile_pool(name="dr", bufs=1, space="DRAM") as dr:
        gsb = pers.tile([1, 194], F32)
        nc.sync.dma_start(out=gsb, in_=g_all)
        besb = pers.tile([1, 194], F32)
        nc.sync.dma_start(out=besb, in_=be_all)
        ones1 = pers.tile([1, 1], F32)
        nc.vector.memset(ones1, 1.0)
        ones8 = pers.tile([8, 1], F32)
        nc.vector.memset(ones8, 0.125)   # 1/8 for mean-of-cores matmul
        epst = pers.tile([1, 1], F32)
        nc.vector.memset(epst, EPS)

        def bn_finish(l, C, regions, bnps, sbp):
            """Cross-core BN: partial stats -> AllGather -> scale/shift [C,1]."""
            # each region is [C, k, <=512]; bn_stats keeps non-innermost dims
            n = sum(r.shape[1] if r.ndim == 3 else 1 for r in regions)
            st = sbp.tile([C, n, 6], F32, tag=f"st{l}")
            i = 0
            for ap in regions:
                k = ap.shape[1] if ap.ndim == 3 else 1
                o = st[:, i:i + k, :] if ap.ndim == 3 else st[:, i, :]
                nc.vector.bn_stats(out=o, in_=ap)
                i += k
            mvt = sbp.tile([C, 2], F32, tag=f"mv{l}")
            nc.vector.bn_aggr(out=mvt, in_=st)
            sti = dr.tile([C, 2], F32, tag=f"sti{l}")
            sto = dr.tile([NCORES, C, 2], F32, tag=f"sto{l}")
            nc.gpsimd.dma_start(out=sti, in_=mvt)
            nc.gpsimd.collective_compute(
                "AllGather", ALU.bypass,
                replica_groups=[list(range(NCORES))],
                ins=[sti.opt()], outs=[sto.opt()])
            G = sbp.tile([8, 2 * C], F32, tag=f"G{l}")
            nc.sync.dma_start(out=G, in_=sto.rearrange("k c two -> k (c two)"))
            Gv = G.rearrange("p (c two) -> p c two", two=2)
            m2 = sbp.tile([8, C], F32, tag=f"m2{l}")
            nc.vector.tensor_mul(m2, Gv[:, :, 0], Gv[:, :, 0])
            pavg = bnps.tile([1, 2 * C], F32, tag="bn")
            nc.tensor.matmul(pavg, ones8, G, start=True, stop=True)
            pavg2 = bnps.tile([1, C], F32, tag="bn")
            nc.tensor.matmul(pavg2, ones8, m2, start=True, stop=True)
            A1 = sbp.tile([1, 2 * C], F32, tag=f"A1{l}")
            nc.scalar.copy(A1, pavg)
            A2 = sbp.tile([1, C], F32, tag=f"A2{l}")
            nc.scalar.copy(A2, pavg2)
            A1v = A1.rearrange("p (c two) -> p c two", two=2)
            am, av = A1v[:, :, 0], A1v[:, :, 1]
            t1 = sbp.tile([1, C], F32, tag=f"t1{l}")
            nc.vector.tensor_mul(t1, am, am)       # E[m]^2
            t2 = sbp.tile([1, C], F32, tag=f"t2{l}")
            nc.vector.tensor_sub(t2, A2, t1)       # Var(means)
            t3 = sbp.tile([1, C], F32, tag=f"t3{l}")
            nc.vector.tensor_add(t3, t2, av)       # + E[var] = total var
            sd = sbp.tile([1, C], F32, tag=f"sd{l}")
            nc.scalar.activation(sd, t3, AF.Sqrt, bias=epst)
            rs = sbp.tile([1, C], F32, tag=f"rs{l}")
            nc.vector.reciprocal(rs, sd)
            off = GOFF[l]
            scl = sbp.tile([1, C], F32, tag=f"scl{l}")
            nc.vector.tensor_mul(scl, gsb[:, off:off + C], rs)
            sh0 = sbp.tile([1, C], F32, tag=f"sh0{l}")
            nc.vector.tensor_mul(sh0, am, scl)
            sh = sbp.tile([1, C], F32, tag=f"sh{l}")
            nc.vector.tensor_sub(sh, besb[:, off:off + C], sh0)
            psc = bnps.tile([C, 1], F32, tag="bn")
            nc.tensor.matmul(psc, scl, ones1, start=True, stop=True)
            psh = bnps.tile([C, 1], F32, tag="bn")
            nc.tensor.matmul(psh, sh, ones1, start=True, stop=True)
            sbs = sbp.tile([C, 1], F32, tag=f"sbs{l}")
            nc.scalar.copy(sbs, psc)
            sbh = sbp.tile([C, 1], F32, tag=f"sbh{l}")
            nc.scalar.copy(sbh, psh)
            return sbs, sbh

        # ---------------- conv phase ----------------
        with tc.tile_pool(name="cb", bufs=1) as cb, \
             tc.tile_pool(name="hp", bufs=2) as hp, \
             tc.tile_pool(name="cps", bufs=6, space="PSUM") as cps, \
             tc.tile_pool(name="bnps", bufs=2, space="PSUM") as bnps:
            x0t = cb.tile([27, YFREE], F32R)
            nc.gpsimd.dma_start(out=x0t, in_=x0)
            mskf = cb.tile([64, 8 * WP], F32)
            nc.gpsimd.dma_start(out=mskf, in_=mask8.partition_broadcast(64))
            mv_ = mskf.rearrange("p (r c) -> p r c", c=WP)
            w0 = cb.tile([27, 64], F32R)
            nc.gpsimd.dma_start(out=w0, in_=w0T)
            wpair, wsing = {}, {}
            for l in (1, 2, 3):
                co = COUT[l]
                for p in range(3):
                    t = cb.tile([128, co], F32R, tag=f"twp{l}{p}")
                    nc.gpsimd.dma_start(out=t, in_=wp_in[l][p])
                    wpair[(l, p)] = t
                    t2 = cb.tile([64, co], F32R, tag=f"tws{l}{p}")
                    nc.gpsimd.dma_start(out=t2, in_=ws_in[l][p])
                    wsing[(l, p)] = t2

            def finish_layer(l, y):
                """BN + ReLU + mask + build padded f32r slab with shifted copy."""
                yv = y.rearrange("p (r c) -> p r c", c=WP)
                regs = [yv[:, r, 1:257] for r in range(4, 36)]
                sbs, sbh = bn_finish(l, 64, regs, bnps, cb)
                h = hp.tile([128, HFREE], F32R, tag="h")
                nc.scalar.activation(h[0:64, LEAD + WP:LEAD + WP + YFREE], y,
                                     AF.Relu, bias=sbh, scale=sbs)
                hv = h[0:64, LEAD + WP:LEAD + WP + YFREE].rearrange(
                    "p (r c) -> p r c", c=WP)
                nc.vector.tensor_mul(hv[:, 0:4, :], hv[:, 0:4, :], mv_[:, 0:4, :])
                nc.vector.tensor_mul(hv[:, 36:40, :], hv[:, 36:40, :], mv_[:, 4:8, :])
                hcv = h[0:64, LEAD + WP:LEAD + WP + YFREE].rearrange(
                    "p (r c) -> p c r", c=WP)
                nc.vector.memset(hcv[:, 0, :].bitcast(F32), 0.0)
                nc.vector.memset(hcv[:, 257, :].bitcast(F32), 0.0)
                nc.vector.memset(h[0:64, 0:LEAD + WP].bitcast(F32), 0.0)
                nc.vector.memset(h[0:64, LEAD + WP + YFREE:HFREE].bitcast(F32), 0.0)
                nc.vector.tensor_copy(h[64:128, 0:HFREE - WP],
                                      h[0:64, WP:HFREE])
                nc.vector.memset(h[64:128, HFREE - WP:HFREE].bitcast(F32), 0.0)
                return h

            # conv0 (im2col input, K=27, one stream)
            if True:
                y = cb.tile([64, YFREE], F32, tag="y")
                for (s, L) in TILES[0]:
                    ps = cps.tile([64, 512], F32, tag="cps")
                    nc.tensor.matmul(ps[:, 0:L], w0, x0t[:, s:s + L],
                                     start=True, stop=True)
                    nc.scalar.copy(y[:, s:s + L], ps[:, 0:L])
                h = finish_layer(0, y)

            # conv1, conv2 (6 streams: 3 pairs K=128 + 3 singles K=64)
            GROUP = 6
            for l in (1, 2):
                y = cb.tile([64, YFREE], F32, tag="y")
                for g0 in range(0, len(TILES[l]), GROUP):
                    grp = TILES[l][g0:g0 + GROUP]
                    pss = [cps.tile([64, 512], F32, tag="cps", name=f"cps{g0}_{i}")
                           for i in range(len(grp))]
                    for p in range(3):
                        for ps, (s, L) in zip(pss, grp):
                            o = LEAD + 516 + s + p - 1
                            nc.tensor.matmul(ps[:, 0:L], wsing[(l, p)],
                                             h[0:64, o:o + L],
                                             start=(p == 0), stop=False)
                    for p in range(3):
                        for ps, (s, L) in zip(pss, grp):
                            o = LEAD + s + p - 1
                            nc.tensor.matmul(ps[:, 0:L], wpair[(l, p)],
                                             h[0:128, o:o + L],
                                             start=False, stop=(p == 2))
                    for ps, (s, L) in zip(pss, grp):
                        nc.scalar.copy(y[:, s:s + L], ps[:, 0:L])
                h = finish_layer(l, y)

            # conv3: output streamed in patch order (gy, py, px, gx)
            def c3rhs(p0, np_, off):
                wide = h[p0:p0 + np_, off:off + 2 * WP]
                w2 = wide.rearrange("p (py c) -> p py c", py=2)
                w3 = w2[:, :, 0:256]
                return w3.rearrange("p py (gx px) -> p py px gx", px=4)

            y3 = cb.tile([2, 8192], F32, tag="y")
            T3 = [(gy, ph) for gy in range(8) for ph in range(2)]
            for g0 in range(0, 16, 6):
                grp = T3[g0:g0 + 6]
                pss = [cps.tile([64, 512], F32, tag="cps", name=f"cps{g0}_{i}")
                           for i in range(len(grp))]
                bases = [LEAD + (5 + 4 * gy + 2 * ph) * WP + 1 for gy, ph in grp]
                for p in range(3):
                    for ps, base in zip(pss, bases):
                        nc.tensor.matmul(ps[0:2, :], wsing[(3, p)],
                                         c3rhs(0, 64, base + WP + (p - 1)),
                                         start=(p == 0), stop=False)
                for p in range(3):
                    for ps, base in zip(pss, bases):
                        nc.tensor.matmul(ps[0:2, :], wpair[(3, p)],
                                         c3rhs(0, 128, base + (p - 1) - WP),
                                         start=False, stop=(p == 2))
                for ps, (gy, ph) in zip(pss, grp):
                    t = gy * 2 + ph
                    nc.scalar.copy(y3[:, t * 512:(t + 1) * 512], ps[0:2, :])
            regs3 = [y3[:, i * 512:(i + 1) * 512] for i in range(16)]
            sbs3, sbh3 = bn_finish(3, 2, regs3, bnps, cb)
            nc.scalar.activation(y3, y3, AF.Relu, bias=sbh3, scale=sbs3)

            # scatter y3 -> patch-major DRAM [16(k=py*4+px), 1024(c,gy,gx)]
            y3p = dr.tile([16, 1024], F32, tag="y3p")
            y5 = y3.rearrange("p (gy py px gx) -> p gy py px gx",
                              gy=8, py=4, px=4)
            y3pr = y3p.rearrange("k (c gy gx) -> k c gy gx", c=2, gy=8)
            for py in range(4):
                for px in range(4):
                    nc.sync.dma_start(out=y3pr[py * 4 + px],
                                      in_=y5[:, :, py, px, :])

        # ---------------- patch augment + AllGather ----------------
        agin = dr.tile([18, 1024], F32, tag="agin")
        gath = dr.tile([8, 18, 1024], F32, tag="gath")
        with tc.tile_pool(name="db", bufs=1) as db, \
             tc.tile_pool(name="sqps", bufs=2, space="PSUM") as sqps:
            Praw = db.tile([16, 1024], F32)
            nc.sync.dma_start(out=Praw, in_=y3p)
            Q = db.tile([16, 1024], F32)
            nc.vector.tensor_mul(Q, Praw, Praw)
            ones16 = db.tile([16, 1], F32)
            nc.vector.memset(ones16, 1.0)
            sqv = db.tile([1, 1024], F32)
            for j in range(2):
                pq = sqps.tile([1, 512], F32, tag="pq")
                nc.tensor.matmul(pq, ones16, Q[:, j * 512:(j + 1) * 512],
                                 start=True, stop=True)
                nc.scalar.copy(sqv[:, j * 512:(j + 1) * 512], pq)
            B16 = db.tile([16, 1024], F32)
            nc.vector.tensor_scalar_mul(B16, Praw, -2.0)
            ones1k = db.tile([1, 1024], F32)
            nc.vector.memset(ones1k, 1.0)
            nc.sync.dma_start(out=agin[0:16, :], in_=B16)
            nc.sync.dma_start(out=agin[16:17, :], in_=ones1k)
            nc.sync.dma_start(out=agin[17:18, :], in_=sqv)
            nc.gpsimd.collective_compute(
                "AllGather", ALU.bypass,
                replica_groups=[list(range(NCORES))],
                ins=[agin.opt()], outs=[gath.opt()])

        # ---------------- distance phase ----------------
        with tc.tile_pool(name="dist", bufs=1) as dist, \
             tc.tile_pool(name="stg", bufs=2) as stg, \
             tc.tile_pool(name="dps", bufs=8, space="PSUM") as dps:
            lhsT = dist.tile([128, 1024], F32)
            nc.sync.dma_start(out=lhsT[0:16, :], in_=agin[0:16, :])
            nc.sync.dma_start(out=lhsT[16:17, :], in_=agin[17:18, :])
            nc.sync.dma_start(out=lhsT[17:18, :], in_=agin[16:17, :])
            nc.vector.tensor_scalar_mul(lhsT[0:16, :], lhsT[0:16, :], -0.5)
            rhs = dist.tile([128, 8192], F32)
            for j in range(16):
                c, k = j // 8, j % 8
                nc.sync.dma_start(out=rhs[0:18, j * 512:(j + 1) * 512],
                                  in_=gath[k, :, c * 512:(c + 1) * 512])
            # replicate the 18 aug rows into 4 PE row-group strips so 4
            # K=18 matmuls run concurrently (tile_position row packing)
            for b in (32, 64, 96):
                nc.vector.tensor_copy(lhsT[b:b + 18, :], lhsT[0:18, :])
                nc.vector.tensor_copy(rhs[b:b + 18, :], rhs[0:18, :])
            for m in range(8):
                stage = stg.tile([128, 8192], F32, tag="stage")
                for n in range(16):
                    b = 32 * ((m * 16 + n) % 4)
                    ps = dps.tile([128, 512], F32, tag="dp")
                    nc.tensor.matmul(ps, lhsT[b:b + 18, m * 128:(m + 1) * 128],
                                     rhs[b:b + 18, n * 512:(n + 1) * 512],
                                     start=True, stop=True,
                                     tile_position=(b, 0))
                    nc.vector.tensor_scalar_max(stage[:, n * 512:(n + 1) * 512],
                                                ps, 0.0)
                    nc.scalar.activation(stage[:, n * 512:(n + 1) * 512],
                                         stage[:, n * 512:(n + 1) * 512], AF.Sqrt)
                nc.sync.dma_start(out=out[m * 128:(m + 1) * 128, :], in_=stage)
    nc.finalize()
    return nc


def _prep_inputs(x, ws_, gs, bes):
    """Per-core numpy input dicts."""
    BF = ml_dtypes.bfloat16
    xp = np.pad(x[0], ((0, 0), (5, 5), (2, 3))).astype(np.float32)
    w0 = ws_[0]
    w0T = np.ascontiguousarray(
        w0.transpose(2, 3, 1, 0).reshape(27, 64)).astype(np.float32)
    wp, wsg = {}, {}
    for l in (1, 2, 3):
        w = ws_[l]
        wp[l] = np.ascontiguousarray(np.stack(
            [np.concatenate([w[:, :, 0, p].T, w[:, :, 1, p].T], 0)
             for p in range(3)])).astype(np.float32)
        wsg[l] = np.ascontiguousarray(np.stack(
            [w[:, :, 2, p].T for p in range(3)])).astype(np.float32)
    g_all = np.concatenate([np.asarray(g, np.float32).ravel() for g in gs]
                           ).reshape(1, 194)
    be_all = np.concatenate([np.asarray(b, np.float32).ravel() for b in bes]
                            ).reshape(1, 194)
    in_maps = []
    for k in range(NCORES):
        col = np.empty((27, ROWS, WP), np.float32)
        for dy in range(3):
            for dx in range(3):
                for ci in range(3):
                    r0 = 32 * k + dy
                    col[(dy * 3 + dx) * 3 + ci] = xp[ci, r0:r0 + ROWS, dx:dx + WP]
        mask = np.zeros((8, WP), np.float32)
        for i, r in enumerate([0, 1, 2, 3, 36, 37, 38, 39]):
            ir = 32 * k - 4 + r
            if 0 <= ir < 256:
                mask[i, 1:257] = 1.0
        in_maps.append(dict(
            x0=np.ascontiguousarray(col.reshape(27, YFREE)),
            w0T=w0T, wp1=wp[1], ws1=wsg[1], wp2=wp[2], ws2=wsg[2],
            wp3=wp[3], ws3=wsg[3], g_all=g_all, be_all=be_all,
            mask8=np.ascontiguousarray(mask.reshape(1, 8 * WP))))
    return in_maps


def kernel(x, w0, b0, g0, be0, w1, b1, g1, be1, w2, b2, g2, be2,
           w3, b3, g3, be3):
    # conv bias b_i cancels exactly inside BatchNorm (mean absorbs it); unused.
    if "nc" not in _CACHE:
        _CACHE["nc"] = build()
    nc = _CACHE["nc"]
    in_maps = _prep_inputs(
        np.asarray(x, np.float32),
        [np.asarray(w, np.float32) for w in (w0, w1, w2, w3)],
        (g0, g1, g2, g3), (be0, be1, be2, be3))
    res = run_bass_kernel_spmd(nc, in_maps, list(range(NCORES)))
    D = np.empty((8192, 8192), np.float32)
    for k in range(NCORES):
        o = res.results[k]["out"]
        for c in range(2):
            D[c * 4096 + k * 512: c * 4096 + (k + 1) * 512, :] = \
                o[c * 512:(c + 1) * 512, :]
    return D
